# revision 1
# baseline (speedup 1.0000x reference)
"""GCN layer (support = x@W; out = D^-1/2 (A_set + I) D^-1/2 support + bias)
distributed across 8 trn2 NeuronCores.

Strategy (row sharding, per sharding hint):
  - Each core owns 1024 output rows (nodes-as-destinations).
  - Host does INDEX-ONLY preprocessing: dedup edges (scatter-set semantics),
    integer degree counts, bucket edges per (core, j-tile, partition) for the
    on-device adjacency build. No float math on host (values are the exact
    structure constants 1.0 / 2.0).
  - Device per core:
      T = D^-1/2 * (x_c @ W)          (hi/lo bf16 split matmuls, fp32 PSUM)
      chunked AllGather of T (packed hi|lo bf16) across the 8 cores
      adjacency tiles [128 j, 1024 i] built on GPSIMD via local_scatter
      out_c = D^-1/2 * (A_c^T.T @ T) + bias   (bf16 hi/lo matmuls, fp32 PSUM)
  - Host concatenates the 8 row blocks.

Precision: all matmul operands are bf16 but split into hi+lo parts
(x, W, T), so the result carries ~2^-16 relative error, far better than
single-pass bf16. Adjacency values {0,1,2} are exact in bf16.
"""

import sys

sys.path.insert(0, "/opt/trn_rl_repo")

import ml_dtypes
import numpy as np

N = 8192
D = 512
CORES = 8
R = N // CORES  # rows per core = 1024
JT = 64  # j-tiles of 128 rows each (in AG-permuted order)
IB = 8  # i-blocks of 128 rows per core
P = 128

_BF16 = np.dtype(ml_dtypes.bfloat16)


def _preprocess(edge_index):
    """Index-only host prep: dedup, degrees, per-core scatter buckets."""
    e0 = np.asarray(edge_index[0], dtype=np.int64)
    e1 = np.asarray(edge_index[1], dtype=np.int64)
    key = e0 * N + e1
    u = np.unique(key)
    i = (u // N).astype(np.int64)
    j = (u % N).astype(np.int64)

    self_mask = i == j
    has_self = np.zeros(N, dtype=np.int64)
    has_self[i[self_mask]] = 1
    i_od = i[~self_mask]
    j_od = j[~self_mask]

    # degree_i = (#distinct neighbors incl. self-edge) + 1 (the added eye)
    deg = np.bincount(i_od, minlength=N).astype(np.int64) + has_self + 1

    # entries: off-diagonal edges (val 1) + diagonal (val 1 or 2)
    diag_i = np.arange(N, dtype=np.int64)
    ent_i = np.concatenate([i_od, diag_i])
    ent_j = np.concatenate([j_od, diag_i])
    ent_v = np.concatenate(
        [np.ones(len(i_od), dtype=np.float32), (1 + has_self).astype(np.float32)]
    )

    core = ent_i // R
    i_loc = ent_i - core * R
    r = ent_j // R
    k = (ent_j % R) // P
    p = ent_j % P
    jt = 8 * k + r  # AG-permuted j-tile index

    g = (core * JT + jt) * P + p  # flat group id, matches [CORES, JT, P] layout
    order = np.argsort(g, kind="stable")
    gs = g[order]
    uniq, first_idx, counts = np.unique(gs, return_index=True, return_counts=True)
    slot = np.arange(len(gs), dtype=np.int64) - np.repeat(first_idx, counts)

    ni = int(counts.max())
    ni = max(2, (ni + 1) // 2 * 2)

    idx_arr = np.full((CORES * JT * P, ni), -1, dtype=np.int16)
    val_arr = np.zeros((CORES * JT * P, ni), dtype=_BF16)
    idx_arr[gs, slot] = i_loc[order].astype(np.int16)
    val_arr[gs, slot] = ent_v[order].astype(_BF16)

    idx_arr = idx_arr.reshape(CORES, JT * P, ni)
    val_arr = val_arr.reshape(CORES, JT * P, ni)
    return idx_arr, val_arr, deg, ni


def _build_nc(
    ni,
    agg_repeat=1,      # repeat the aggregation phase (timing experiments)
    agg_loop=0,        # >0: wrap aggregation in a For_i loop of this count
                       # (output stays correct: each iteration re-accumulates
                       # from start=True). Used for slope-based timing.
    use_scatter=True,  # False: memset adjacency tiles (wrong result, timing)
    use_ag=True,       # False: local DMA copy instead of AllGather (1-core sim)
    hilo=True,         # False: single-pass bf16 (lower precision, faster)
    f32r=False,        # aggregation in float32r (implies hilo=False path)
    kernel_repeat=1,   # repeat the whole kernel (steady-state timing; output
                       # stays correct — every repeat recomputes identically)
    agg_bufs=3,
    num_devices=CORES,
):
    from concourse import bacc, mybir, tile

    f32 = mybir.dt.float32
    bf16 = mybir.dt.bfloat16
    i16 = mybir.dt.int16
    mult = mybir.AluOpType.mult
    sub = mybir.AluOpType.subtract
    add = mybir.AluOpType.add

    nc = bacc.Bacc(
        "TRN2", target_bir_lowering=False, debug=False, num_devices=num_devices
    )

    xT_d = nc.dram_tensor("xT", [D, R], f32, kind="ExternalInput")
    w_d = nc.dram_tensor("w", [D, D], f32, kind="ExternalInput")
    bias_d = nc.dram_tensor("bias", [1, D], f32, kind="ExternalInput")
    deg_d = nc.dram_tensor("deg", [P, IB], f32, kind="ExternalInput")
    eidx_d = nc.dram_tensor("eidx", [JT * P, ni], i16, kind="ExternalInput")
    eval_d = nc.dram_tensor("eval", [JT * P, ni], bf16, kind="ExternalInput")
    y_d = nc.dram_tensor("y", [R, D], f32, kind="ExternalOutput")

    RG = [list(range(num_devices))]
    KC = D // P  # 4 k-chunks of the feature dim
    if f32r:
        hilo = False
    # packed T layout: hi|lo bf16 pairs (hilo and f32r modes) or single bf16
    t_dt = bf16
    PW = 2 * D if (hilo or f32r) else D
    f32r_dt = mybir.dt.float32r

    with tile.TileContext(nc) as tc:
        with (
            tc.tile_pool(name="const", bufs=1) as const_pool,
            tc.tile_pool(name="xt", bufs=1) as xt_pool,
            tc.tile_pool(name="sup", bufs=2) as sup_pool,
            tc.tile_pool(name="tpack", bufs=2) as tpack_pool,
            tc.tile_pool(name="agg", bufs=agg_bufs) as agg_pool,
            tc.tile_pool(name="out", bufs=3) as out_pool,
            tc.tile_pool(name="acc", bufs=1, space="PSUM") as acc_pool,
            tc.tile_pool(name="dram", bufs=1, space="DRAM") as dram_pool,
        ):
            # ---- constants ----
            bias_bc = const_pool.tile([P, D], f32, tag="bias_bc")
            nc.sync.dma_start(bias_bc[:], bias_d.ap().to_broadcast((P, D)))

            degt = const_pool.tile([P, IB], f32, tag="degt")
            nc.sync.dma_start(degt[:], deg_d.ap())
            dsq = const_pool.tile([P, IB], f32, tag="dsq")
            nc.scalar.activation(dsq[:], degt[:], mybir.ActivationFunctionType.Sqrt)
            dis = const_pool.tile([P, IB], f32, tag="dis")
            nc.vector.reciprocal(dis[:], dsq[:])

            # W split into bf16 hi/lo, per k-chunk [128, 512]
            wh, wl = [], []
            for kc in range(KC):
                wf = sup_pool.tile([P, D], f32, tag="wf")
                nc.sync.dma_start(wf[:], w_d.ap()[kc * P : (kc + 1) * P, :])
                h = const_pool.tile([P, D], bf16, tag=f"wh{kc}")
                nc.vector.tensor_copy(out=h[:], in_=wf[:])
                l = const_pool.tile([P, D], bf16, tag=f"wl{kc}")
                nc.vector.tensor_tensor(out=l[:], in0=wf[:], in1=h[:], op=sub)
                wh.append(h)
                wl.append(l)

            # xT split into bf16 hi/lo, per k-chunk [128, 1024]
            xth, xtl = [], []
            for kc in range(KC):
                xf = xt_pool.tile([P, R], f32, tag=f"xf{kc}")
                nc.sync.dma_start(xf[:], xT_d.ap()[kc * P : (kc + 1) * P, :])
                h = xt_pool.tile([P, R], bf16, tag=f"xth{kc}")
                nc.vector.tensor_copy(out=h[:], in_=xf[:])
                l = xt_pool.tile([P, R], bf16, tag=f"xtl{kc}")
                nc.vector.tensor_tensor(out=l[:], in0=xf[:], in1=h[:], op=sub)
                xth.append(h)
                xtl.append(l)

            # PSUM accumulators, one bank per i-block
            acc = [
                acc_pool.tile([P, D], f32, tag=f"acc{b}", name=f"acc{b}")
                for b in range(IB)
            ]

            # internal DRAM for the chunked AllGather (per kernel-repeat:
            # a Shared tile may only have one writer)
            agin_r = [
                [
                    dram_pool.tile(
                        [P, PW], t_dt, tag=f"agin{b}_{kr}", name=f"agin{b}_{kr}"
                    )
                    for b in range(IB)
                ]
                for kr in range(kernel_repeat)
            ]
            tchunk_r = [
                [
                    dram_pool.tile(
                        [CORES * P, PW], t_dt,
                        tag=f"tchunk{k}_{kr}", name=f"tchunk{k}_{kr}",
                        addr_space="Shared" if use_ag else "Local",
                    )
                    for k in range(IB)
                ]
                for kr in range(kernel_repeat)
            ]

            import contextlib

            for krep in range(kernel_repeat):
                agin = agin_r[krep]
                tchunk = tchunk_r[krep]
                # -- phase 1: support = x_c @ W (hi/lo), scale, pack, gather --
                for b in range(IB):
                    bs = slice(b * P, (b + 1) * P)
                    # emit in kc-major order for weight locality
                    emit = []
                    for kc in range(KC):
                        emit += [
                            (xth[kc], wh[kc]),
                            (xth[kc], wl[kc]),
                            (xtl[kc], wh[kc]),
                        ]
                    for ti, (xt_t, w_t) in enumerate(emit):
                        nc.tensor.matmul(
                            acc[b][:],
                            lhsT=xt_t[:, bs],
                            rhs=w_t[:],
                            start=(ti == 0),
                            stop=(ti == len(emit) - 1),
                        )

                    tf = sup_pool.tile([P, D], f32, tag="tf")
                    nc.vector.tensor_scalar(
                        out=tf[:], in0=acc[b][:], scalar1=dis[:, b : b + 1],
                        scalar2=None, op0=mult,
                    )
                    tp = tpack_pool.tile([P, PW], t_dt, tag="tp")
                    nc.vector.tensor_copy(out=tp[:, 0:D], in_=tf[:])
                    if hilo or f32r:
                        nc.vector.tensor_tensor(
                            out=tp[:, D : 2 * D], in0=tf[:], in1=tp[:, 0:D], op=sub
                        )
                    nc.sync.dma_start(agin[b][:], tp[:])
                    if use_ag:
                        nc.gpsimd.collective_compute(
                            "AllGather",
                            mybir.AluOpType.bypass,
                            replica_groups=RG,
                            ins=[agin[b].opt()],
                            outs=[tchunk[b].opt()],
                        )
                    else:
                        for rr in range(CORES):
                            nc.sync.dma_start(
                                tchunk[b][rr * P : (rr + 1) * P, :], agin[b][:]
                            )

                # -- phase 2: aggregation out += A_cT.T @ [T_hi | T_lo] --
                loop_cm = (
                    tc.For_i(0, agg_loop, 1)
                    if agg_loop > 0
                    else contextlib.nullcontext()
                )
                with loop_cm:
                    for rep in range(agg_repeat):
                        for jt in range(JT):
                            k, r = jt // 8, jt % 8
                            tt = agg_pool.tile([P, PW], t_dt, tag="tt")
                            nc.sync.dma_start(
                                tt[:], tchunk[k][r * P : (r + 1) * P, :]
                            )
                            ei = agg_pool.tile([P, ni], i16, tag="ei")
                            nc.sync.dma_start(
                                ei[:], eidx_d.ap()[jt * P : (jt + 1) * P, :]
                            )
                            ev = agg_pool.tile([P, ni], bf16, tag="ev")
                            nc.sync.dma_start(
                                ev[:], eval_d.ap()[jt * P : (jt + 1) * P, :]
                            )

                            at = agg_pool.tile([P, R], bf16, tag="at")
                            if use_scatter:
                                nc.gpsimd.local_scatter(at[:], ev[:], ei[:], P, R, ni)
                            else:
                                nc.vector.memset(at[:], 0.0)

                            if f32r:
                                atx = agg_pool.tile([P, R], f32r_dt, tag="atx")
                                nc.vector.tensor_copy(out=atx[:], in_=at[:])
                                lhs_tile = atx
                                ttx = agg_pool.tile([P, D], f32r_dt, tag="ttx")
                                nc.vector.tensor_tensor(
                                    out=ttx[:], in0=tt[:, 0:D], in1=tt[:, D : 2 * D],
                                    op=add,
                                )
                                rhs_ap = ttx[:]
                            else:
                                lhs_tile = at
                                rhs_ap = tt[:, 0:D]

                            start = rep == 0 and jt == 0
                            last = rep == agg_repeat - 1 and jt == JT - 1
                            for b in range(IB):
                                lhs = lhs_tile[:, b * P : (b + 1) * P]
                                nc.tensor.matmul(
                                    acc[b][:], lhsT=lhs, rhs=rhs_ap,
                                    start=start, stop=(last and not hilo),
                                )
                                if hilo:
                                    nc.tensor.matmul(
                                        acc[b][:], lhsT=lhs, rhs=tt[:, D : 2 * D],
                                        start=False, stop=last,
                                    )

                # -- phase 3: scale + bias + store --
                for b in range(IB):
                    yf = out_pool.tile([P, D], f32, tag="yf")
                    nc.vector.tensor_scalar(
                        out=yf[:], in0=acc[b][:], scalar1=dis[:, b : b + 1],
                        scalar2=None, op0=mult,
                    )
                    yo = out_pool.tile([P, D], f32, tag="yo")
                    nc.vector.tensor_tensor(
                        out=yo[:], in0=yf[:], in1=bias_bc[:], op=add
                    )
                    nc.sync.dma_start(y_d.ap()[b * P : (b + 1) * P, :], yo[:])

    nc.compile()
    return nc


def kernel(x, edge_index, weight, bias):
    from concourse import bass_utils

    x = np.asarray(x, dtype=np.float32)
    weight = np.asarray(weight, dtype=np.float32)
    bias = np.asarray(bias, dtype=np.float32)

    idx_arr, val_arr, deg, ni = _preprocess(edge_index)

    nc = _build_nc(ni)

    in_maps = []
    for c in range(CORES):
        rows = slice(c * R, (c + 1) * R)
        xT_c = np.ascontiguousarray(x[rows].T)
        deg_c = np.ascontiguousarray(
            deg[rows].astype(np.float32).reshape(IB, P).T
        )
        in_maps.append(
            {
                "xT": xT_c,
                "w": weight,
                "bias": bias.reshape(1, D),
                "deg": deg_c,
                "eidx": idx_arr[c],
                "eval": val_arr[c],
            }
        )

    res = bass_utils.run_bass_kernel_spmd(
        nc, in_maps, core_ids=list(range(CORES)), trace=False
    )
    kernel.last_results = res
    kernel.last_nc = nc
    kernel.last_in_maps = in_maps

    out = np.concatenate([res.results[c]["y"] for c in range(CORES)], axis=0)
    return out



# revision 3
# speedup vs baseline: 929.7736x; 929.7736x over previous
"""GCN layer (support = x@W; out = D^-1/2 (A_set + I) D^-1/2 support + bias)
distributed across 8 trn2 NeuronCores.

Strategy (row sharding, per sharding hint):
  - Each core owns 1024 output rows (nodes-as-destinations).
  - Host does INDEX-ONLY preprocessing: dedup edges (scatter-set semantics),
    integer degree counts, bucket edges per (core, j-tile, partition) for the
    on-device adjacency build. No float math on host (values are the exact
    structure constants 1.0 / 2.0).
  - Device per core:
      T = D^-1/2 * (x_c @ W)          (hi/lo bf16 split matmuls, fp32 PSUM)
      chunked AllGather of T (packed hi|lo bf16) across the 8 cores
      adjacency tiles [128 j, 1024 i] built on GPSIMD via local_scatter
      out_c = D^-1/2 * (A_c^T.T @ T) + bias   (bf16 hi/lo matmuls, fp32 PSUM)
  - Host concatenates the 8 row blocks.

Precision: all matmul operands are bf16 but split into hi+lo parts
(x, W, T), so the result carries ~2^-16 relative error, far better than
single-pass bf16. Adjacency values {0,1,2} are exact in bf16.
"""

import sys

sys.path.insert(0, "/opt/trn_rl_repo")

import ml_dtypes
import numpy as np

N = 8192
D = 512
CORES = 8
R = N // CORES  # rows per core = 1024
JT = 64  # j-tiles of 128 rows each (in AG-permuted order)
IB = 8  # i-blocks of 128 rows per core
P = 128

_BF16 = np.dtype(ml_dtypes.bfloat16)


def _preprocess(edge_index):
    """Index-only host prep: dedup, degrees, per-core scatter buckets."""
    e0 = np.asarray(edge_index[0], dtype=np.int64)
    e1 = np.asarray(edge_index[1], dtype=np.int64)
    key = e0 * N + e1
    u = np.unique(key)
    i = (u // N).astype(np.int64)
    j = (u % N).astype(np.int64)

    self_mask = i == j
    has_self = np.zeros(N, dtype=np.int64)
    has_self[i[self_mask]] = 1
    i_od = i[~self_mask]
    j_od = j[~self_mask]

    # degree_i = (#distinct neighbors incl. self-edge) + 1 (the added eye)
    deg = np.bincount(i_od, minlength=N).astype(np.int64) + has_self + 1

    # entries: off-diagonal edges (val 1) + diagonal (val 1 or 2)
    diag_i = np.arange(N, dtype=np.int64)
    ent_i = np.concatenate([i_od, diag_i])
    ent_j = np.concatenate([j_od, diag_i])
    ent_v = np.concatenate(
        [np.ones(len(i_od), dtype=np.float32), (1 + has_self).astype(np.float32)]
    )

    core = ent_i // R
    i_loc = ent_i - core * R
    r = ent_j // R
    k = (ent_j % R) // P
    p = ent_j % P
    jt = 8 * k + r  # AG-permuted j-tile index

    g = (core * JT + jt) * P + p  # flat group id, matches [CORES, JT, P] layout
    order = np.argsort(g, kind="stable")
    gs = g[order]
    uniq, first_idx, counts = np.unique(gs, return_index=True, return_counts=True)
    slot = np.arange(len(gs), dtype=np.int64) - np.repeat(first_idx, counts)

    ni = int(counts.max())
    ni = max(2, (ni + 1) // 2 * 2)

    idx_arr = np.full((CORES * JT * P, ni), -1, dtype=np.int16)
    val_arr = np.zeros((CORES * JT * P, ni), dtype=_BF16)
    idx_arr[gs, slot] = i_loc[order].astype(np.int16)
    val_arr[gs, slot] = ent_v[order].astype(_BF16)

    idx_arr = idx_arr.reshape(CORES, JT * P, ni)
    val_arr = val_arr.reshape(CORES, JT * P, ni)
    return idx_arr, val_arr, deg, ni


def _build_nc(
    ni,
    agg_repeat=1,      # repeat the aggregation phase (timing experiments)
    agg_loop=0,        # >0: wrap aggregation in a For_i loop of this count
                       # (output stays correct: each iteration re-accumulates
                       # from start=True). Used for slope-based timing.
    use_scatter=True,  # False: memset adjacency tiles (wrong result, timing)
    use_ag=True,       # False: local DMA copy instead of AllGather (1-core sim)
    hilo=True,         # False: single-pass bf16 (lower precision, faster)
    f32r=False,        # aggregation in float32r (implies hilo=False path)
    kernel_repeat=1,   # repeat the whole kernel (steady-state timing; output
                       # stays correct — every repeat recomputes identically)
    agg_bufs=3,
    num_devices=CORES,
):
    from concourse import bacc, mybir, tile

    f32 = mybir.dt.float32
    bf16 = mybir.dt.bfloat16
    i16 = mybir.dt.int16
    mult = mybir.AluOpType.mult
    sub = mybir.AluOpType.subtract
    add = mybir.AluOpType.add

    nc = bacc.Bacc(
        "TRN2", target_bir_lowering=False, debug=False, num_devices=num_devices
    )

    xT_d = nc.dram_tensor("xT", [D, R], f32, kind="ExternalInput")
    w_d = nc.dram_tensor("w", [D, D], f32, kind="ExternalInput")
    bias_d = nc.dram_tensor("bias", [1, D], f32, kind="ExternalInput")
    deg_d = nc.dram_tensor("deg", [P, IB], f32, kind="ExternalInput")
    eidx_d = nc.dram_tensor("eidx", [JT * P, ni], i16, kind="ExternalInput")
    eval_d = nc.dram_tensor("eval", [JT * P, ni], bf16, kind="ExternalInput")
    y_d = nc.dram_tensor("y", [R, D], f32, kind="ExternalOutput")

    RG = [list(range(num_devices))]
    KC = D // P  # 4 k-chunks of the feature dim
    if f32r:
        hilo = False
    # packed T layout: hi|lo bf16 pairs (hilo and f32r modes) or single bf16
    t_dt = bf16
    PW = 2 * D if (hilo or f32r) else D
    f32r_dt = mybir.dt.float32r

    with tile.TileContext(nc) as tc:
        with (
            tc.tile_pool(name="const", bufs=1) as const_pool,
            tc.tile_pool(name="xt", bufs=1) as xt_pool,
            tc.tile_pool(name="sup", bufs=2) as sup_pool,
            tc.tile_pool(name="tpack", bufs=2) as tpack_pool,
            tc.tile_pool(name="agg", bufs=agg_bufs) as agg_pool,
            tc.tile_pool(name="out", bufs=3) as out_pool,
            tc.tile_pool(name="acc", bufs=1, space="PSUM") as acc_pool,
            tc.tile_pool(name="dram", bufs=1, space="DRAM") as dram_pool,
        ):
            # ---- constants ----
            bias_bc = const_pool.tile([P, D], f32, tag="bias_bc")
            nc.sync.dma_start(bias_bc[:], bias_d.ap().to_broadcast((P, D)))

            degt = const_pool.tile([P, IB], f32, tag="degt")
            nc.sync.dma_start(degt[:], deg_d.ap())
            dsq = const_pool.tile([P, IB], f32, tag="dsq")
            nc.scalar.activation(dsq[:], degt[:], mybir.ActivationFunctionType.Sqrt)
            dis = const_pool.tile([P, IB], f32, tag="dis")
            nc.vector.reciprocal(dis[:], dsq[:])

            # W split into bf16 hi/lo, per k-chunk [128, 512]
            wh, wl = [], []
            for kc in range(KC):
                wf = sup_pool.tile([P, D], f32, tag="wf")
                nc.sync.dma_start(wf[:], w_d.ap()[kc * P : (kc + 1) * P, :])
                h = const_pool.tile([P, D], bf16, tag=f"wh{kc}")
                nc.vector.tensor_copy(out=h[:], in_=wf[:])
                l = const_pool.tile([P, D], bf16, tag=f"wl{kc}")
                nc.vector.tensor_tensor(out=l[:], in0=wf[:], in1=h[:], op=sub)
                wh.append(h)
                wl.append(l)

            # xT split into bf16 hi/lo, per k-chunk [128, 1024]
            xth, xtl = [], []
            for kc in range(KC):
                xf = xt_pool.tile([P, R], f32, tag=f"xf{kc}")
                nc.sync.dma_start(xf[:], xT_d.ap()[kc * P : (kc + 1) * P, :])
                h = xt_pool.tile([P, R], bf16, tag=f"xth{kc}")
                nc.vector.tensor_copy(out=h[:], in_=xf[:])
                l = xt_pool.tile([P, R], bf16, tag=f"xtl{kc}")
                nc.vector.tensor_tensor(out=l[:], in0=xf[:], in1=h[:], op=sub)
                xth.append(h)
                xtl.append(l)

            # PSUM accumulators, one bank per i-block
            acc = [
                acc_pool.tile([P, D], f32, tag=f"acc{b}", name=f"acc{b}")
                for b in range(IB)
            ]

            # internal DRAM for the chunked AllGather (per kernel-repeat:
            # a Shared tile may only have one writer)
            agin_r = [
                [
                    dram_pool.tile(
                        [P, PW], t_dt, tag=f"agin{b}_{kr}", name=f"agin{b}_{kr}"
                    )
                    for b in range(IB)
                ]
                for kr in range(kernel_repeat)
            ]
            tchunk_r = [
                [
                    dram_pool.tile(
                        [CORES * P, PW], t_dt,
                        tag=f"tchunk{k}_{kr}", name=f"tchunk{k}_{kr}",
                        addr_space="Shared" if use_ag else "Local",
                    )
                    for k in range(IB)
                ]
                for kr in range(kernel_repeat)
            ]

            import contextlib

            for krep in range(kernel_repeat):
                agin = agin_r[krep]
                tchunk = tchunk_r[krep]
                # -- phase 1: support = x_c @ W (hi/lo), scale, pack, gather --
                for b in range(IB):
                    bs = slice(b * P, (b + 1) * P)
                    # emit in kc-major order for weight locality
                    emit = []
                    for kc in range(KC):
                        emit += [
                            (xth[kc], wh[kc]),
                            (xth[kc], wl[kc]),
                            (xtl[kc], wh[kc]),
                        ]
                    for ti, (xt_t, w_t) in enumerate(emit):
                        nc.tensor.matmul(
                            acc[b][:],
                            lhsT=xt_t[:, bs],
                            rhs=w_t[:],
                            start=(ti == 0),
                            stop=(ti == len(emit) - 1),
                        )

                    tf = sup_pool.tile([P, D], f32, tag="tf")
                    nc.vector.tensor_scalar(
                        out=tf[:], in0=acc[b][:], scalar1=dis[:, b : b + 1],
                        scalar2=None, op0=mult,
                    )
                    tp = tpack_pool.tile([P, PW], t_dt, tag="tp")
                    nc.vector.tensor_copy(out=tp[:, 0:D], in_=tf[:])
                    if hilo or f32r:
                        nc.vector.tensor_tensor(
                            out=tp[:, D : 2 * D], in0=tf[:], in1=tp[:, 0:D], op=sub
                        )
                    nc.sync.dma_start(agin[b][:], tp[:])
                    if use_ag:
                        nc.gpsimd.collective_compute(
                            "AllGather",
                            mybir.AluOpType.bypass,
                            replica_groups=RG,
                            ins=[agin[b].opt()],
                            outs=[tchunk[b].opt()],
                        )
                    else:
                        for rr in range(CORES):
                            nc.sync.dma_start(
                                tchunk[b][rr * P : (rr + 1) * P, :], agin[b][:]
                            )

                # -- phase 2: aggregation out += A_cT.T @ [T_hi | T_lo] --
                loop_cm = (
                    tc.For_i(0, agg_loop, 1)
                    if agg_loop > 0
                    else contextlib.nullcontext()
                )
                with loop_cm:
                    for rep in range(agg_repeat):
                        for jt in range(JT):
                            k, r = jt // 8, jt % 8
                            tt = agg_pool.tile([P, PW], t_dt, tag="tt")
                            nc.sync.dma_start(
                                tt[:], tchunk[k][r * P : (r + 1) * P, :]
                            )
                            ei = agg_pool.tile([P, ni], i16, tag="ei")
                            nc.sync.dma_start(
                                ei[:], eidx_d.ap()[jt * P : (jt + 1) * P, :]
                            )
                            ev = agg_pool.tile([P, ni], bf16, tag="ev")
                            nc.sync.dma_start(
                                ev[:], eval_d.ap()[jt * P : (jt + 1) * P, :]
                            )

                            at = agg_pool.tile([P, R], bf16, tag="at")
                            if use_scatter:
                                nc.gpsimd.local_scatter(at[:], ev[:], ei[:], P, R, ni)
                            else:
                                nc.vector.memset(at[:], 0.0)

                            if f32r:
                                atx = agg_pool.tile([P, R], f32r_dt, tag="atx")
                                nc.vector.tensor_copy(out=atx[:], in_=at[:])
                                lhs_tile = atx
                                ttx = agg_pool.tile([P, D], f32r_dt, tag="ttx")
                                nc.vector.tensor_tensor(
                                    out=ttx[:], in0=tt[:, 0:D], in1=tt[:, D : 2 * D],
                                    op=add,
                                )
                                rhs_ap = ttx[:]
                            else:
                                lhs_tile = at
                                rhs_ap = tt[:, 0:D]

                            start = rep == 0 and jt == 0
                            last = rep == agg_repeat - 1 and jt == JT - 1
                            for b in range(IB):
                                lhs = lhs_tile[:, b * P : (b + 1) * P]
                                nc.tensor.matmul(
                                    acc[b][:], lhsT=lhs, rhs=rhs_ap,
                                    start=start, stop=(last and not hilo),
                                )
                                if hilo:
                                    nc.tensor.matmul(
                                        acc[b][:], lhsT=lhs, rhs=tt[:, D : 2 * D],
                                        start=False, stop=last,
                                    )

                # -- phase 3: scale + bias + store --
                for b in range(IB):
                    yf = out_pool.tile([P, D], f32, tag="yf")
                    nc.vector.tensor_scalar(
                        out=yf[:], in0=acc[b][:], scalar1=dis[:, b : b + 1],
                        scalar2=None, op0=mult,
                    )
                    yo = out_pool.tile([P, D], f32, tag="yo")
                    nc.vector.tensor_tensor(
                        out=yo[:], in0=yf[:], in1=bias_bc[:], op=add
                    )
                    nc.sync.dma_start(y_d.ap()[b * P : (b + 1) * P, :], yo[:])

    nc.compile()
    return nc


def kernel(x, edge_index, weight, bias):
    from concourse import bass_utils

    x = np.asarray(x, dtype=np.float32)
    weight = np.asarray(weight, dtype=np.float32)
    bias = np.asarray(bias, dtype=np.float32)

    idx_arr, val_arr, deg, ni = _preprocess(edge_index)

    # Single-pass bf16 everywhere: rel err ~3.1e-3 (CPU-emulated) vs the
    # 2e-2 gate; halves AG bytes, tchunk traffic, and phase-2 PE time
    # relative to the hi/lo split.
    nc = _build_nc(ni, hilo=False)

    in_maps = []
    for c in range(CORES):
        rows = slice(c * R, (c + 1) * R)
        xT_c = np.ascontiguousarray(x[rows].T)
        deg_c = np.ascontiguousarray(
            deg[rows].astype(np.float32).reshape(IB, P).T
        )
        in_maps.append(
            {
                "xT": xT_c,
                "w": weight,
                "bias": bias.reshape(1, D),
                "deg": deg_c,
                "eidx": idx_arr[c],
                "eval": val_arr[c],
            }
        )

    res = bass_utils.run_bass_kernel_spmd(
        nc, in_maps, core_ids=list(range(CORES)), trace=False
    )
    kernel.last_results = res
    kernel.last_nc = nc
    kernel.last_in_maps = in_maps
    kernel.last_ni = ni

    out = np.concatenate([res.results[c]["y"] for c in range(CORES)], axis=0)
    return out

